# revision 5
# baseline (speedup 1.0000x reference)
"""Trainium2 Bass kernel for nn_AttentionModel (sparse_attention).

Reference computation per batch b (B=128, N=1024, E=512, H=8, DK=64, S=1):
  glimpse_Q = reshape(query)                       # [H,B,1,DK]
  compat[h,n] = q_h . K[h,n] / sqrt(DK),  masked -> -inf
  attn = softmax_n(compat)
  heads[h] = sum_n attn[h,n] V[h,n]                # [H,DK]
  glimpse = concat_h(heads) @ W_out.T              # [E]
  logits[n] = tanh(glimpse . logit_K[n] / sqrt(E)) * 10, masked -> -inf

Strategy: pure data parallel over B across 8 cores (16 batches/core).
On-chip layout puts all (h,b) pairs on the 128 SBUF partitions so every
vector op uses all lanes; K/V/logit_K stream through SBUF in large
contiguous-per-partition chunks (memory-bound regime).  Softmax is computed
online (flash-style) so the K and V passes fuse into one streamed phase.
"""

import numpy as np

import concourse.bacc as bacc
import concourse.mybir as mybir
import concourse.tile as tile
from concourse.bass_utils import run_bass_kernel_spmd

B, N, E, H, DK = 128, 1024, 512, 8, 64
NCORES = 8
BL = B // NCORES          # batches per core
P = 128                   # SBUF partitions = H * BL
NCH = 64                  # n-chunk for the K/V stream
NCHUNKS = N // NCH        # 16
TL = 8                    # n rows per partition in the logits phase (N / P)
NEG_BIG = -3.4e38
F32 = mybir.dt.float32
U8 = mybir.dt.uint8

# engine assignment knobs (True -> GPSIMD/Pool, False -> DVE)
K_TT_ON_POOL = [c % 16 < 7 for c in range(NCHUNKS)]
V_TT_ON_POOL = [True] * NCHUNKS
LK_TT_ON_POOL = [b % 2 == 0 for b in range(BL)]
# logits reduce: "ts" (DVE tensor_scalar accum), "act" (ScalarE accum)
LK_RED = ["ts" if b < 10 else "act" for b in range(BL)]

_CACHE = {}


def _build():
    nc = bacc.Bacc("TRN2", target_bir_lowering=False, debug=False,
                   num_devices=NCORES)
    k_d = nc.dram_tensor("k_hb", [P, N, DK], F32, kind="ExternalInput").ap()
    v_d = nc.dram_tensor("v_hb", [P, N, DK], F32, kind="ExternalInput").ap()
    lk_d = nc.dram_tensor("lk", [BL, N, E], F32, kind="ExternalInput").ap()
    q_d = nc.dram_tensor("q_hb", [P, DK], F32, kind="ExternalInput").ap()
    wt_d = nc.dram_tensor("w_t", [E, E], F32, kind="ExternalInput").ap()
    mask_d = nc.dram_tensor("mask_u8", [BL, N], U8, kind="ExternalInput").ap()
    basis_d = nc.dram_tensor("basis", [BL, BL * P], F32, kind="ExternalInput").ap()
    ident_d = nc.dram_tensor("ident", [P, P], F32, kind="ExternalInput").ap()

    logits_o = nc.dram_tensor("logits_o", [BL, N], F32, kind="ExternalOutput").ap()
    glimpse_o = nc.dram_tensor("glimpse_o", [BL, E], F32, kind="ExternalOutput").ap()

    inv_sqrt_dk = float(1.0 / np.sqrt(np.float32(DK)))
    inv_sqrt_e = float(1.0 / np.sqrt(np.float32(E)))

    with tile.TileContext(nc) as tc:
        with (
            tc.tile_pool(name="state", bufs=1) as st,
            tc.tile_pool(name="kpool", bufs=3) as kp,
            tc.tile_pool(name="vpool", bufs=3) as vp,
            tc.tile_pool(name="lkpool", bufs=3) as lkp,
            tc.tile_pool(name="small", bufs=3) as sm,
            tc.tile_pool(name="gbp", bufs=2) as gbp,
            tc.tile_pool(name="ps_t", bufs=1, space="PSUM") as ps_t,
            tc.tile_pool(name="ps_g", bufs=1, space="PSUM") as ps_g,
            tc.tile_pool(name="ps_b", bufs=2, space="PSUM") as ps_b,
        ):
            # ---- constants / persistent state ----
            q_sb = st.tile([P, DK], F32)
            nc.sync.dma_start(out=q_sb[:], in_=q_d)
            ident_sb = st.tile([P, P], F32)
            nc.sync.dma_start(out=ident_sb[:], in_=ident_d)
            basis_sb = st.tile([BL, BL * P], F32)
            nc.sync.dma_start(out=basis_sb[:], in_=basis_d)
            wt_sb = st.tile([P, 4, E], F32)
            nc.sync.dma_start(out=wt_sb[:], in_=wt_d.rearrange("(c p) e -> p c e", p=P))

            mask_sb = st.tile([P, N], U8)
            for h in range(H):
                nc.sync.dma_start(out=mask_sb[h * BL:(h + 1) * BL, :], in_=mask_d)
            maskneg = st.tile([P, N], F32)
            nc.vector.tensor_scalar_mul(maskneg[:], mask_sb[:], NEG_BIG)

            mask_lg = st.tile([P, BL, TL], U8)
            nc.sync.dma_start(out=mask_lg[:],
                              in_=mask_d.rearrange("b (p t) -> p b t", t=TL))

            m_a = st.tile([P, 1], F32)
            m_b = st.tile([P, 1], F32)
            l_run = st.tile([P, 1], F32)
            heads = st.tile([P, DK], F32)
            nc.vector.memset(m_a[:], NEG_BIG)
            nc.vector.memset(l_run[:], 0.0)
            nc.vector.memset(heads[:], 0.0)

            # ---- fused K/V streaming phase with online softmax ----
            m_old, m_new = m_a, m_b
            for c in range(NCHUNKS):
                n0 = c * NCH
                k_tile = kp.tile([P, NCH, DK], F32)
                nc.sync.dma_start(out=k_tile[:], in_=k_d[:, n0:n0 + NCH, :])
                qb = q_sb[:].unsqueeze(1).broadcast_to([P, NCH, DK])
                eng = nc.gpsimd if K_TT_ON_POOL[c] else nc.vector
                eng.tensor_mul(k_tile[:], k_tile[:], qb)

                s_c = sm.tile([P, NCH], F32)
                nc.vector.reduce_sum(out=s_c[:], in_=k_tile[:],
                                     axis=mybir.AxisListType.X)
                nc.vector.tensor_add(s_c[:], s_c[:], maskneg[:, n0:n0 + NCH])

                mc = sm.tile([P, 1], F32)
                nc.vector.reduce_max(out=mc[:], in_=s_c[:],
                                     axis=mybir.AxisListType.X)
                nc.vector.tensor_max(m_new[:], m_old[:], mc[:])
                dm = sm.tile([P, 1], F32)
                nc.vector.tensor_sub(dm[:], m_old[:], m_new[:])
                alpha = sm.tile([P, 1], F32)
                nc.scalar.activation(out=alpha[:], in_=dm[:],
                                     func=mybir.ActivationFunctionType.Exp,
                                     scale=inv_sqrt_dk)
                bias = sm.tile([P, 1], F32)
                nc.vector.tensor_scalar_mul(bias[:], m_new[:], -inv_sqrt_dk)

                p_c = sm.tile([P, NCH], F32)
                ls = sm.tile([P, 1], F32)
                nc.scalar.activation(out=p_c[:], in_=s_c[:],
                                     func=mybir.ActivationFunctionType.Exp,
                                     bias=bias[:], scale=inv_sqrt_dk,
                                     accum_out=ls[:])
                nc.vector.tensor_scalar(out=l_run[:], in0=l_run[:],
                                        scalar1=alpha[:], scalar2=ls[:],
                                        op0=mybir.AluOpType.mult,
                                        op1=mybir.AluOpType.add)
                nc.vector.tensor_scalar_mul(heads[:], heads[:], alpha[:])

                v_tile = vp.tile([P, NCH, DK], F32)
                nc.sync.dma_start(out=v_tile[:], in_=v_d[:, n0:n0 + NCH, :])
                pb = p_c[:].unsqueeze(2).broadcast_to([P, NCH, DK])
                eng = nc.gpsimd if V_TT_ON_POOL[c] else nc.vector
                eng.tensor_mul(v_tile[:], v_tile[:], pb)
                hp = sm.tile([P, DK], F32)
                nc.vector.reduce_sum(out=hp[:], in_=v_tile[:].transpose([0, 2, 1]),
                                     axis=mybir.AxisListType.X)
                nc.vector.tensor_add(heads[:], heads[:], hp[:])

                m_old, m_new = m_new, m_old

            r_run = st.tile([P, 1], F32)
            nc.vector.reciprocal(out=r_run[:], in_=l_run[:])
            nc.vector.tensor_scalar_mul(heads[:], heads[:], r_run[:])

            # ---- projection: glimpse = heads_cat @ W_out.T ----
            t1_ps = ps_t.tile([DK, P], F32)
            nc.tensor.transpose(t1_ps[:], heads[:], ident_sb[:])
            t1_sb = st.tile([DK, P], F32)
            nc.scalar.copy(out=t1_sb[:], in_=t1_ps[:])

            g_ps = ps_g.tile([BL, E], F32)
            lhs_tiles = []
            for cc in range(4):
                lhsT = st.tile([P, BL], F32, tag=f"lhsT{cc}")
                for h2 in range(2):
                    hh = 2 * cc + h2
                    nc.scalar.copy(out=lhsT[h2 * DK:(h2 + 1) * DK, :],
                                   in_=t1_sb[:, hh * BL:(hh + 1) * BL])
                lhs_tiles.append(lhsT)
            for cc in range(4):
                nc.tensor.matmul(out=g_ps[:], lhsT=lhs_tiles[cc][:],
                                 rhs=wt_sb[:, cc, :],
                                 start=(cc == 0), stop=(cc == 3))
            glimpse_sb = st.tile([BL, E], F32)
            nc.scalar.copy(out=glimpse_sb[:], in_=g_ps[:])
            nc.sync.dma_start(out=glimpse_o, in_=glimpse_sb[:])

            # ---- logits phase ----
            logits_all = st.tile([P, BL, TL], F32)
            for b in range(BL):
                lk_tile = lkp.tile([P, TL, E], F32)
                nc.sync.dma_start(out=lk_tile[:],
                                  in_=lk_d[b].rearrange("(p t) e -> p t e", t=TL))
                gb_ps = ps_b.tile([P, E], F32)
                nc.tensor.matmul(out=gb_ps[:],
                                 lhsT=basis_sb[:, b * P:(b + 1) * P],
                                 rhs=glimpse_sb[:], start=True, stop=True)
                gb_sb = gbp.tile([P, E], F32)
                nc.scalar.copy(out=gb_sb[:], in_=gb_ps[:])
                gbb = gb_sb[:].unsqueeze(1).broadcast_to([P, TL, E])
                eng = nc.gpsimd if LK_TT_ON_POOL[b] else nc.vector
                eng.tensor_mul(lk_tile[:], lk_tile[:], gbb)
                if LK_RED[b] == "ts":
                    for t in range(TL):
                        nc.vector.tensor_scalar(
                            out=lk_tile[:, t, :], in0=lk_tile[:, t, :],
                            scalar1=1.0, scalar2=0.0,
                            op0=mybir.AluOpType.mult, op1=mybir.AluOpType.add,
                            accum_out=logits_all[:, b, t:t + 1])
                else:
                    for t in range(TL):
                        nc.scalar.activation(
                            out=lk_tile[:, t, :], in_=lk_tile[:, t, :],
                            func=mybir.ActivationFunctionType.Copy,
                            accum_out=logits_all[:, b, t:t + 1])

            # ---- tail: tanh * 10, mask -> -inf, store ----
            lt = st.tile([P, BL, TL], F32)
            nc.scalar.activation(out=lt[:], in_=logits_all[:],
                                 func=mybir.ActivationFunctionType.Tanh,
                                 scale=inv_sqrt_e)
            nc.vector.tensor_scalar_mul(lt[:], lt[:], 10.0)
            neginf = st.tile([P, BL, TL], F32)
            nc.vector.memset(neginf[:], float("-inf"))
            nc.vector.copy_predicated(out=lt[:], mask=mask_lg[:], data=neginf[:])
            nc.sync.dma_start(out=logits_o.rearrange("b (p t) -> p b t", t=TL),
                              in_=lt[:])

    nc.compile()
    return nc


def _get_nc():
    if "nc" not in _CACHE:
        _CACHE["nc"] = _build()
    return _CACHE["nc"]


def _prep_in_maps(query, glimpse_K, glimpse_V, logit_K, W_out, mask):
    q = np.ascontiguousarray(np.asarray(query, dtype=np.float32)).reshape(B, E)
    gk = np.ascontiguousarray(np.asarray(glimpse_K, dtype=np.float32)).reshape(H, B, N, DK)
    gv = np.ascontiguousarray(np.asarray(glimpse_V, dtype=np.float32)).reshape(H, B, N, DK)
    lk = np.ascontiguousarray(np.asarray(logit_K, dtype=np.float32)).reshape(B, N, E)
    wt = np.ascontiguousarray(np.asarray(W_out, dtype=np.float32).T)
    mu8 = np.ascontiguousarray(np.asarray(mask)).reshape(B, N).view(np.uint8)

    basis = np.zeros((BL, BL * P), dtype=np.float32)
    for b in range(BL):
        basis[b, b * P:(b + 1) * P] = 1.0
    ident = np.eye(P, dtype=np.float32)

    in_maps = []
    for c in range(NCORES):
        b0, b1 = c * BL, (c + 1) * BL
        in_maps.append(dict(
            k_hb=np.ascontiguousarray(gk[:, b0:b1]).reshape(P, N, DK),
            v_hb=np.ascontiguousarray(gv[:, b0:b1]).reshape(P, N, DK),
            lk=np.ascontiguousarray(lk[b0:b1]),
            q_hb=np.ascontiguousarray(
                q[b0:b1].reshape(BL, H, DK).transpose(1, 0, 2)).reshape(P, DK),
            w_t=wt,
            mask_u8=np.ascontiguousarray(mu8[b0:b1]),
            basis=basis,
            ident=ident,
        ))
    return in_maps


def run_sharded(inputs, trace=False, trace_kwargs=None):
    """Run on 8 cores; returns ((logits, glimpse), BassKernelResults)."""
    nc = _get_nc()
    in_maps = _prep_in_maps(**inputs)
    kw = {}
    if trace:
        kw["trace"] = True
        if trace_kwargs:
            kw["trace_kwargs"] = trace_kwargs
    res = run_bass_kernel_spmd(nc, in_maps, core_ids=list(range(NCORES)), **kw)
    logits = np.empty((B, 1, N), dtype=np.float32)
    glimpse = np.empty((B, 1, E), dtype=np.float32)
    for c in range(NCORES):
        b0, b1 = c * BL, (c + 1) * BL
        logits[b0:b1, 0, :] = res.results[c]["logits_o"]
        glimpse[b0:b1, 0, :] = res.results[c]["glimpse_o"]
    return (logits, glimpse), res


def kernel(query, glimpse_K, glimpse_V, logit_K, W_out, mask):
    (logits, glimpse), _ = run_sharded(dict(
        query=query, glimpse_K=glimpse_K, glimpse_V=glimpse_V,
        logit_K=logit_K, W_out=W_out, mask=mask))
    return logits, glimpse


class JitRunner:
    """Reusable jitted multi-core runner (device-resident inputs, no
    per-call recompile) for benchmarking.  Mirrors the multi-core branch of
    bass2jax.run_bass_via_pjrt."""

    def __init__(self, inputs):
        import jax
        from jax.experimental.shard_map import shard_map
        from jax.sharding import Mesh, NamedSharding, PartitionSpec

        from concourse import bass2jax, mybir as _mybir

        self.jax = jax
        nc = _get_nc()
        in_maps = _prep_in_maps(**inputs)
        bass2jax.install_neuronx_cc_hook()

        partition_name = (nc.partition_id_tensor.name
                          if nc.partition_id_tensor else None)
        in_names, out_names, out_avals, zero_outs = [], [], [], []
        for alloc in nc.m.functions[0].allocations:
            if not isinstance(alloc, _mybir.MemoryLocationSet):
                continue
            name = alloc.memorylocations[0].name
            if alloc.kind == "ExternalInput":
                if name != partition_name:
                    in_names.append(name)
            elif alloc.kind == "ExternalOutput":
                shape = tuple(alloc.tensor_shape)
                dtype = _mybir.dt.np(alloc.dtype)
                out_names.append(name)
                out_avals.append(jax.core.ShapedArray(shape, dtype))
                zero_outs.append(np.zeros(shape, dtype))
        self.out_names = out_names
        n_params = len(in_names)
        all_in_names = in_names + out_names
        if partition_name is not None:
            all_in_names = all_in_names + [partition_name]

        def _body(*args):
            operands = list(args)
            if partition_name is not None:
                operands.append(bass2jax.partition_id_tensor())
            outs = bass2jax._bass_exec_p.bind(
                *operands,
                out_avals=tuple(out_avals),
                in_names=tuple(all_in_names),
                out_names=tuple(out_names),
                lowering_input_output_aliases=(),
                sim_require_finite=True,
                sim_require_nnan=True,
                nc=nc,
            )
            return tuple(outs)

        devices = jax.devices()[:NCORES]
        mesh = Mesh(np.asarray(devices), ("core",))
        spec = PartitionSpec("core")
        self.fn = jax.jit(
            shard_map(_body, mesh=mesh,
                      in_specs=(spec,) * (n_params + len(out_names)),
                      out_specs=(spec,) * len(out_names),
                      check_rep=False),
            keep_unused=True,
        )
        sh = NamedSharding(mesh, spec)
        self.args = [
            jax.device_put(
                np.concatenate([np.asarray(in_maps[c][nm]) for c in range(NCORES)],
                               axis=0), sh)
            for nm in in_names
        ] + [
            jax.device_put(np.concatenate([z] * NCORES, axis=0), sh)
            for z in zero_outs
        ]

    def run(self):
        return self.fn(*self.args)

    def time(self, n_iter=20, n_warm=3):
        import time as _t
        for _ in range(n_warm):
            o = self.run()
        self.jax.block_until_ready(o)
        t0 = _t.perf_counter()
        outs = [self.run() for _ in range(n_iter)]
        self.jax.block_until_ready(outs)
        return (_t.perf_counter() - t0) / n_iter

    def outputs(self):
        o = self.run()
        self.jax.block_until_ready(o)
        res = {nm: np.asarray(a) for nm, a in zip(self.out_names, o)}
        logits = res["logits_o"].reshape(B, 1, N)
        glimpse = res["glimpse_o"].reshape(B, 1, E)
        return logits, glimpse
